# revision 28
# baseline (speedup 1.0000x reference)
"""Trainium2 Bass kernel for ConvMessageAggregator (fp16, DVE/ACT balanced).

Computes, for each node n (messages: [N, 16, 688] fp32):
  f1[i] = relu(w10*x[i] + w11*x[i+2] + b1)      i in 0..13   (dilated 2-tap conv)
  f2[i] = relu(w20*f1[i] + w21*f1[i+2] + b2)    i in 0..11
  out   = relu(sum_k mlp_w[k] * f2[6+k] + mlp_b)             -> [N, 688]

Only x rows 6..15 are consumed; the host stages those 10 rows as fp16
(rel err ~1.5e-3 vs the 2e-2 gate) and upcasts the fp16 result to fp32.

Engine plan (ACT is always 1x on TRN2; DVE TT 2x / tensor_scalar 4x with
fused 2-op (op0,const0)(op1,const1); GpSimd tensor ops are slow (0.42 eff)
AND degrade concurrent DVE 2-port modes, so Pool is left idle):
  DVE  A1: t = TSP(x_ot, mult r1, add c1)  then  B1: t += x_pv (TT)
  ACT  C1: f1 = Relu(p1*t) on the 6 conv2-pv rows only (in place)
  DVE  sc4 = |r2| * f1[shared 4 rows]            (single-op TSP 4x)
  ACT  sc2 = Relu(|r2|*p1 * t[2 non-pv rows])    (pre-relu legs)
  DVE  B2: u = f1_pv -/+ sc (TT; conv2 scale pass eliminated entirely)
  C2:  G_k = w_k*relu(p2*u_k + b2):
    ACT rows: Relu(scale=|w_k|p2, bias=|w_k|b2)  (sign resolved in tree)
    DVE rows: TSP(mult w_k*p2, max/min -w_k*b2) = G_k - w_k*b2; the
      constant w_k*b2 is folded into the final bias
  DVE  tree: pairwise sign-merge TT add/sub; final TSP(add bias, max 0)
Balanced ~18.5us/tile x 8 tiles on both engines (~150us busy); DMA ~95us
fully overlapped. HW exec ~180us (run-to-run clock variance +-10%).

A hand-authored 2X_1PORT custom-DVE op for the conv inner pass (the T1
mechanism) compiles and generates tables here, but this container's
walrus rejects InstCustomDveAnt at codegen ("ISA wrong length" version
skew), so USE_CUSTOM_AFF stays off.
"""

import sys

for _p in ("/opt/trn_rl_repo",):
    if _p not in sys.path:
        sys.path.insert(0, _p)

import numpy as np

import concourse.bass as bass
import concourse.tile as tile
from concourse import mybir
from concourse.bass_utils import run_bass_kernel_spmd

N_FULL, L, MSG = 16384, 16, 688
N_CORES = 8
N_LOCAL = N_FULL // N_CORES  # 2048
P = 128
TW = 2                        # node blocks per tile
NTILES = N_LOCAL // (P * TW)  # 8
R0, NROWS = 6, 10
FB = TW * MSG                 # fused (block, msg) free dim

F16 = mybir.dt.float16
F32 = mybir.dt.float32
AF = mybir.ActivationFunctionType
OP = mybir.AluOpType

USE_CUSTOM_AFF = False  # walrus build rejects InstCustomDveAnt (ISA wrong length)
PERF_MAX_ON = False
USE_STOCK_AFF = True
N_C2_DVE = 0   # signed G rows on DVE in steady state
N_SC_DVE = 1   # sc legs on DVE


# --------------------------------------------------------------------------
# Custom 2x DVE op: out = (Src0*C0 + C1) + Src1
# --------------------------------------------------------------------------

def _aff2x_2x_uop():
    """Hand-authored 2X_1PORT program (T1 mechanism from the custom-dve
    design doc): lo element on ALU blocks 0-2, hi element on blocks 3-5
    reading the *_HI packed lanes, results packed to WR0_LO/WR0_HI."""
    from concourse.dve_uop import (
        ENABLE,
        AluInp,
        AluOp,
        DelayInp,
        InpSel,
        OutPath,
        OutSel,
        Trigger,
        UopConfig,
    )

    u = UopConfig()
    u.enable_input(InpSel.SRC_0, 0)
    u.enable_input(InpSel.SRC_1, 1)      # chain 0
    u.enable_input(InpSel.SRC_0_HI, 2)   # chain 1
    u.enable_input(InpSel.SRC_1_HI, 3)   # chain 2
    u.enable_input(InpSel.CONST_0, 4)    # chain 3 (r)
    u.enable_input(InpSel.CONST_1, 5)    # chain 4 (c)
    u.require_inp0 = ENABLE
    u.require_inp1 = ENABLE
    u.trigger = (Trigger.SRC_TENSOR_DONE, Trigger.NONE, Trigger.NONE)
    dp = u.datapath_config
    # b0: lo1 = SRC_0 * C0
    dp[0].enable_alu(AluOp.MULTIPLY, AluInp.PREV_ALU_OUT, AluInp.PREV_DELAY_3)
    dp[0].pass_through_delay(0, 1, 2, 3, 4)
    # b1: lo2 = lo1 + C1
    dp[1].enable_alu(AluOp.ADD, AluInp.PREV_ALU_OUT, AluInp.PREV_DELAY_4)
    dp[1].pass_through_delay(0, 1, 2, 3, 4)
    # b2: lo = lo2 + SRC_1
    dp[2].enable_alu(AluOp.ADD, AluInp.PREV_ALU_OUT, AluInp.PREV_DELAY_0)
    dp[2].pass_through_delay(1, 2, 3, 4)
    # b3: hi1 = SRC_0_HI * C0; park lo in chain 0
    dp[3].enable_alu(AluOp.MULTIPLY, AluInp.PREV_DELAY_1, AluInp.PREV_DELAY_3)
    dp[3].enable_delay_from_src(DelayInp.PREV_ALU_OUT, 0)
    dp[3].pass_through_delay(2, 4)
    # b4: hi2 = hi1 + C1
    dp[4].enable_alu(AluOp.ADD, AluInp.PREV_ALU_OUT, AluInp.PREV_DELAY_4)
    dp[4].pass_through_delay(0, 2)
    # b5: hi = hi2 + SRC_1_HI
    dp[5].enable_alu(AluOp.ADD, AluInp.PREV_ALU_OUT, AluInp.PREV_DELAY_2)
    dp[5].pass_through_delay(0)
    # b6: ALU <- lo (from chain 0); park hi in chain 1
    dp[6].enable_alu(AluOp.BYPASS, AluInp.PREV_DELAY_0)
    dp[6].enable_delay_from_src(DelayInp.PREV_ALU_OUT, 1)
    # b7: ALU <- lo; carry hi
    dp[7].enable_alu(AluOp.BYPASS, AluInp.PREV_ALU_OUT)
    dp[7].pass_through_delay(1)
    u.enable_output(OutSel.ALU_OUT, OutPath.WR0_LO)
    u.enable_output(OutSel.DELAY_1, OutPath.WR0_HI)
    u.validate("v3")
    return u


_AFF_OP = None


def _get_aff_op():
    """Register CONV_AFF_2X in the dve_ops registry (runtime append - the
    same operation the official flow does in-source) and seed the compile
    cache with a DveOpSpec carrying the hand-built 2x variant."""
    global _AFF_OP
    if _AFF_OP is not None:
        return _AFF_OP
    import concourse.dve_ops as dom
    from concourse.dve_spec import C0, C1, Spec, Src0, Src1
    from concourse.dve_spec import lower as dve_lower
    from concourse.dve_uop import DveOpSpec

    name = "CONV_AFF_2X"
    spec = Spec(
        body=(Src0 * C0 + C1) + Src1,
        reference=lambda in0, in1, s0, s1, imm2: (
            in0.astype(np.float32) * s0 + s1
        )
        + in1,
    )
    op = dom.DveOp(name, spec, subdim=False, uops_sha={})
    row = max(dom._SUB_OPCODE_FOR_NAME.values()) + 1
    assert row < 0x20, "no free custom-DVE opcode rows"
    dom.OPS.append(op)
    dom._SUB_OPCODE_FOR_NAME[name] = row
    dom.CUSTOM_DVE_SPECS[name] = spec
    uops_1x = dve_lower(spec, ver="v3")
    dspec = DveOpSpec(
        name=name,
        opcode=row,
        uops=uops_1x,
        uops_2x=[_aff2x_2x_uop()],
        perf_max=1,
        rd1_en=True,
    )
    dspec.validate("v3")
    dom._COMPILE_CACHE[(name, "v3")] = dspec
    _AFF_OP = op
    return op


def _split_multi_waits(nc):
    """TPB instructions encode at most ONE semaphore wait; this walrus build's
    codegen rejects instructions with more. Hoist extra waits into standalone
    EventSemaphore ops on the same (in-order) sequencer."""
    for func in nc.m.functions:
        for bb in func.blocks:
            insts = list(bb.instructions)
            if not any(
                i.sync_info is not None and len(i.sync_info.on_wait) > 1
                for i in insts
            ):
                continue
            new = []
            for inst in insts:
                si = inst.sync_info
                if si is not None and len(si.on_wait) > 1:
                    waits = list(si.on_wait)
                    for j, w in enumerate(waits[:-1]):
                        new.append(
                            mybir.InstEventSemaphore(
                                name=f"{inst.name}-hoistw{j}",
                                engine=inst.engine,
                                sync_info=mybir.SyncInfo(on_wait=[w], on_update=[]),
                            )
                        )
                    inst.sync_info = mybir.SyncInfo(
                        on_wait=[waits[-1]], on_update=list(si.on_update)
                    )
                new.append(inst)
            bb.instructions = new


def _enable_aff_perf(nc):
    """codegen packs byte36 = row | rd1_en<<5 | perf_max<<6; _custom_dve
    leaves perf_max=0 (mode Disable). Request 2X_1PORT; the engine verifies
    the mem-pattern and falls back to 1x if it doesn't qualify."""
    for func in nc.m.functions:
        for bb in func.blocks:
            for inst in bb.instructions:
                if type(inst).__name__ == "InstCustomDveAnt":
                    inst.perf_max = 1


def _conv_split(wa, wb, b):
    """Factor pre[i] = wa*in[i] + wb*in[i+2] + b as p*(in[pv] + r*in[ot] + c)
    with |r| <= 1 (pv = dominant tap), p = dominant weight, c = b/p."""
    if abs(wa) >= abs(wb):
        p, r, pv, ot = wa, (wb / wa if wa != 0.0 else 0.0), 0, 2
    else:
        p, r, pv, ot = wb, wa / wb, 2, 0
    c = b / p if p != 0.0 else 0.0
    return p, r, c, pv, ot


def build_program(w10, w11, b1, w20, w21, b2, mlp_w, mlp_b):
    nc = bass.Bass(trn_type="TRN2", name="conv_msg_agg")
    x = nc.dram_tensor("x", [N_LOCAL, NROWS, MSG], F16, kind="ExternalInput")
    out = nc.dram_tensor("out", [N_LOCAL, MSG], F16, kind="ExternalOutput")

    p1, r1, c1, pv1, ot1 = _conv_split(w10, w11, b1)
    p2, r2, c2, pv2, ot2 = _conv_split(w20, w21, b2)
    ar2 = abs(r2)
    b2op = OP.add if r2 > 0 else OP.subtract
    nzk = [k for k in range(6) if mlp_w[k] != 0.0]
    scheme_z = p1 != 0.0 and p2 != 0.0 and bool(nzk)
    # floater routing: DVE gets the final op, N_C2_DVE signed G rows (largest
    # |w| first, also guarantees a sign-+ tree root) and N_SC_DVE sc rows.
    by_mag = sorted(nzk, key=lambda k: -abs(mlp_w[k]))
    n_c2_dve = N_C2_DVE
    if nzk and not any(mlp_w[k] > 0 for k in nzk):
        n_c2_dve = max(1, N_C2_DVE)  # need a sign-+ tree root
    dve_rows = set(by_mag[:n_c2_dve]) if nzk else set()
    # conv2 bias recovered via w*relu(z+b2) = max/min(w*p2*u, -w*b2) + w*b2;
    # the constant lands in the final bias.
    fin_bias = mlp_b + sum(mlp_w[k] * b2 for k in dve_rows)

    with tile.TileContext(nc) as tc:
        with (
            tc.tile_pool(name="bias", bufs=1) as pool_b,
            tc.tile_pool(name="xin", bufs=2) as pool_x,
            tc.tile_pool(name="v1p", bufs=2) as pool_1,
            tc.tile_pool(name="v1sp", bufs=2) as pool_1s,
            tc.tile_pool(name="v2p", bufs=3) as pool_2,
            tc.tile_pool(name="outp", bufs=4) as pool_o,
        ):
            fbias = pool_b.tile([P, 1], F32, tag="fb", name="fb")
            nc.vector.memset(fbias[:], fin_bias)
            gbias = {}
            for k in nzk:
                if k not in dve_rows and abs(mlp_w[k]) * b2 != 0.0:
                    gbias[k] = pool_b.tile([P, 1], F32, tag=f"gb{k}", name=f"gb{k}")
                    nc.vector.memset(gbias[k][:], abs(mlp_w[k]) * b2)
            for it in range(NTILES):
                n0 = it * TW * P
                xt = pool_x.tile([P, NROWS, TW, MSG], F16, tag="x")
                front_split = it <= 1
                back_split = it == NTILES - 1
                if it == 0:
                    blks = [(0, 1, 0, MSG // 2), (0, 1, MSG // 2, MSG)] + [
                        (blk, blk + 1, 0, MSG) for blk in range(1, TW)
                    ]
                elif front_split or back_split:
                    blks = [(blk, blk + 1, 0, MSG) for blk in range(TW)]
                else:
                    blks = [(0, TW, 0, MSG)]
                bblks = (
                    [(blk, blk + 1) for blk in range(TW)]
                    if back_split
                    else [(0, TW)]
                )
                for lo, hi, m0, m1 in blks:
                    nc.sync.dma_start(
                        out=xt[:, :, lo:hi, m0:m1],
                        in_=x[n0 + lo * P : n0 + hi * P, :, m0:m1].rearrange(
                            "(b p) r m -> p r b m", b=hi - lo
                        ),
                    )

                # conv1 pre: t = (r1*x_ot + c1) + x_pv  (TSP 4x + TT 2x)
                v1 = pool_1.tile([P, 8, TW, MSG], F16, tag="v1")
                if p1 == 0.0:
                    nc.vector.memset(v1[:], max(b1, 0.0) / (p1 if p1 else 1.0))
                else:
                    for lo, hi, m0, m1 in blks:
                        nc.vector.tensor_scalar(
                            out=v1[:, :, lo:hi, m0:m1],
                            in0=xt[:, ot1 : ot1 + 8, lo:hi, m0:m1],
                            scalar1=r1, scalar2=c1,
                            op0=OP.mult, op1=OP.add,
                        )
                        nc.vector.tensor_tensor(
                            out=v1[:, :, lo:hi, m0:m1],
                            in0=v1[:, :, lo:hi, m0:m1],
                            in1=xt[:, pv1 : pv1 + 8, lo:hi, m0:m1],
                            op=OP.add,
                        )
                        # raw relu in place on the conv2 pv rows only;
                        # DVE in the drain phase (ACT would starve DVE)
                        if scheme_z and back_split and lo >= TW - 1:
                            nc.vector.tensor_scalar(
                                out=v1[:, pv2 : pv2 + 6, lo:hi, m0:m1],
                                in0=v1[:, pv2 : pv2 + 6, lo:hi, m0:m1],
                                scalar1=p1, scalar2=0.0,
                                op0=OP.mult, op1=OP.max,
                            )
                        elif scheme_z:
                            nc.scalar.activation(
                                out=v1[:, pv2 : pv2 + 6, lo:hi, m0:m1],
                                in_=v1[:, pv2 : pv2 + 6, lo:hi, m0:m1],
                                func=AF.Relu, bias=0.0, scale=p1,
                            )
                        else:
                            nc.scalar.activation(
                                out=v1[:, :, lo:hi, m0:m1],
                                in_=v1[:, :, lo:hi, m0:m1],
                                func=AF.Relu, bias=0.0, scale=p1,
                            )

                v1s = pool_1s.tile([P, 6, TW, MSG], F16, tag="v1s")
                v2 = pool_2.tile([P, 6, TW, MSG], F16, tag="v2")
                ot = pool_o.tile([P, TW, MSG], F16, tag="o")
                for lo, hi in bblks:
                    if not scheme_z:
                        # degenerate weights: conservative fallback
                        base = max(b1, 0.0) if p1 == 0.0 else None
                        if p2 == 0.0 or not nzk:
                            fb = max(b2, 0.0) if p2 == 0.0 else 0.0
                            tval = sum(mlp_w[k] * fb for k in nzk) + mlp_b
                            nc.vector.memset(ot[:, lo:hi, :], max(tval, 0.0))
                        else:
                            # p1 == 0: f1 rows are the constant max(b1,0)
                            f1c = max(b1, 0.0)
                            zc = w20 * f1c + w21 * f1c + b2
                            tval = sum(
                                mlp_w[k] * max(zc, 0.0) for k in nzk
                            ) + mlp_b
                            nc.vector.memset(ot[:, lo:hi, :], max(tval, 0.0))
                        nc.sync.dma_start(
                            out=out[n0 + lo * P : n0 + hi * P].rearrange(
                                "(b p) m -> p b m", b=hi - lo
                            ),
                            in_=ot[:, lo:hi, :],
                        )
                        continue

                    # conv1 sc legs: sc_j = |r2| * relu(p1 * t[ot2+j]).
                    # The 4 rows shared with the raw range are already
                    # relu'd in v1 -> one single-op 4x TSP mult on DVE;
                    # the 2 non-shared rows come from pre-relu t -> one
                    # batched ACT Relu.
                    sh0 = max(pv2, ot2)          # first shared f1 row
                    j0 = sh0 - ot2               # its v1s row
                    nc.vector.tensor_scalar_mul(
                        v1s[:, j0 : j0 + 4, lo:hi, :],
                        v1[:, sh0 : sh0 + 4, lo:hi, :],
                        ar2,
                    )
                    t0 = ot2 if ot2 < pv2 else ot2 + 4  # non-shared rows
                    jt = t0 - ot2
                    if back_split and lo >= TW - 1:
                        nc.vector.tensor_scalar(
                            out=v1s[:, jt : jt + 2, lo:hi, :],
                            in0=v1[:, t0 : t0 + 2, lo:hi, :],
                            scalar1=ar2 * p1, scalar2=0.0,
                            op0=OP.mult, op1=OP.max,
                        )
                    else:
                        nc.scalar.activation(
                            out=v1s[:, jt : jt + 2, lo:hi, :],
                            in_=v1[:, t0 : t0 + 2, lo:hi, :],
                            func=AF.Relu, bias=0.0, scale=ar2 * p1,
                        )

                    # conv2 pre: u = f1_raw_pv +/- sc  (TT 2x)
                    nc.vector.tensor_tensor(
                        out=v2[:, :, lo:hi, :],
                        in0=v1[:, pv2 : pv2 + 6, lo:hi, :],
                        in1=v1s[:, :, lo:hi, :],
                        op=b2op,
                    )

                    # C2: G_k = w_k * relu(p2*u_k + b2). In the drain
                    # phase (last tile) ACT-routed rows would starve DVE,
                    # so route everything to DVE there.
                    drain = (back_split and lo >= TW - 1) or it == 0
                    bblk_dve = dve_rows if not drain else set(nzk)
                    bblk_fin = fin_bias if not drain else (
                        mlp_b + sum(mlp_w[k] * b2 for k in nzk)
                    )
                    terms = []  # (sign, row_ap), sign-+ rows first
                    for k in nzk:
                        dst = v2[:, k, lo:hi, :]
                        if k in bblk_dve:
                            nc.vector.tensor_scalar(
                                out=dst, in0=dst,
                                scalar1=mlp_w[k] * p2,
                                scalar2=-mlp_w[k] * b2,
                                op0=OP.mult,
                                op1=OP.max if mlp_w[k] > 0 else OP.min,
                            )
                            terms.insert(0, (1, dst))
                        else:
                            nc.scalar.activation(
                                out=dst, in_=dst, func=AF.Relu,
                                bias=gbias[k][:] if k in gbias else 0.0,
                                scale=abs(mlp_w[k]) * p2,
                            )
                            terms.append((1 if mlp_w[k] > 0 else -1, dst))

                    # tree level-1: if all 3 adjacent pairs need the SAME
                    # op, batch them into one strided TT (out rows 0,2,4).
                    # A reversed subtract pair just flips the result sign,
                    # which the tag tracks (tag = in0 row's sign).
                    if len(nzk) == 6:
                        # effective tags: DVE rows hold signed values (tag +)
                        s = [1 if (k in bblk_dve or mlp_w[k] > 0) else -1
                             for k in range(6)]
                        pair_ops = [OP.add if s[2*i] == s[2*i+1] else OP.subtract
                                    for i in range(3)]
                        batched = pair_ops[0] == pair_ops[1] == pair_ops[2]
                    else:
                        batched = False
                    if batched:
                        # strided 3-row APs must stay <=2 free dims for the
                        # 2x packed mode: fuse (blk, msg) when contiguous
                        if lo == 0 and hi == TW:
                            fl = lambda ap: ap.rearrange("p r b m -> p r (b m)")
                        else:
                            fl = lambda ap: ap.rearrange("p r b m -> p (r b) m")
                        nc.vector.tensor_tensor(
                            out=fl(v2[:, 0:6:2, lo:hi, :]),
                            in0=fl(v2[:, 0:6:2, lo:hi, :]),
                            in1=fl(v2[:, 1:6:2, lo:hi, :]),
                            op=pair_ops[0],
                        )
                        terms = [(s[2*i], v2[:, 2*i, lo:hi, :]) for i in range(3)]
                        terms.sort(key=lambda t: -t[0])
                    # pairwise sign-merge (TT add/sub on DVE)
                    while len(terms) > 1:
                        pos = [t for t in terms if t[0] > 0]
                        neg = [t for t in terms if t[0] < 0]
                        if len(neg) >= 2:
                            (sa, aa), (sb, ab) = neg[0], neg[1]
                            op = OP.add
                        elif len(pos) >= 2 and len(neg) == 0:
                            (sa, aa), (sb, ab) = pos[0], pos[1]
                            op = OP.add
                        else:  # fold the last negative into a positive
                            (sa, aa), (sb, ab) = pos[0], neg[0]
                            op = OP.subtract
                        nc.vector.tensor_tensor(out=aa, in0=aa, in1=ab, op=op)
                        terms = [
                            t for t in terms if t[1] is not aa and t[1] is not ab
                        ]
                        terms.insert(0, (sa, aa))

                    # final: out = Relu(T + bias); ACT normally, DVE in
                    # the drain phase (survivor sign + either way)
                    assert terms[0][0] > 0
                    if drain:
                        nc.vector.tensor_scalar(
                            out=ot[:, lo:hi, :], in0=terms[0][1],
                            scalar1=bblk_fin, scalar2=0.0,
                            op0=OP.add, op1=OP.max,
                        )
                    else:
                        nc.scalar.activation(
                            out=ot[:, lo:hi, :], in_=terms[0][1],
                            func=AF.Relu, bias=fbias[:], scale=1.0,
                        )
                    nc.sync.dma_start(
                        out=out[n0 + lo * P : n0 + hi * P].rearrange(
                            "(b p) m -> p b m", b=hi - lo
                        ),
                        in_=ot[:, lo:hi, :],
                    )
    _split_multi_waits(nc)
    return nc


def run(inputs, trace=False, **spmd_kwargs):
    """Build + run on 8 cores. Returns (full_output, BassKernelResults)."""
    msgs = np.asarray(inputs["messages"])
    assert msgs.shape == (N_FULL, L, MSG), msgs.shape
    xs = np.ascontiguousarray(msgs[:, R0 : R0 + NROWS, :], dtype=np.float16)

    c1w = np.asarray(inputs["conv1_w"], dtype=np.float64)
    c2w = np.asarray(inputs["conv2_w"], dtype=np.float64)
    mlw = np.asarray(inputs["mlp_w"], dtype=np.float64)
    nc = build_program(
        float(c1w[0]),
        float(c1w[1]),
        float(np.asarray(inputs["conv1_b"], dtype=np.float64)),
        float(c2w[0]),
        float(c2w[1]),
        float(np.asarray(inputs["conv2_b"], dtype=np.float64)),
        [float(v) for v in mlw],
        float(np.asarray(inputs["mlp_b"], dtype=np.float64)),
    )

    in_maps = [
        {"x": xs[i * N_LOCAL : (i + 1) * N_LOCAL]} for i in range(N_CORES)
    ]
    res = run_bass_kernel_spmd(
        nc, in_maps, core_ids=list(range(N_CORES)), trace=trace, **spmd_kwargs
    )
    full = np.concatenate([r["out"] for r in res.results], axis=0).astype(
        np.float32
    )
    return full, res


def kernel(**inputs) -> np.ndarray:
    return run(inputs, trace=False)[0]


# revision 29
# speedup vs baseline: 1.0218x; 1.0218x over previous
"""Trainium2 Bass kernel for ConvMessageAggregator (fp16, DVE/ACT balanced).

Computes, for each node n (messages: [N, 16, 688] fp32):
  f1[i] = relu(w10*x[i] + w11*x[i+2] + b1)      i in 0..13   (dilated 2-tap conv)
  f2[i] = relu(w20*f1[i] + w21*f1[i+2] + b2)    i in 0..11
  out   = relu(sum_k mlp_w[k] * f2[6+k] + mlp_b)             -> [N, 688]

Only x rows 6..15 are consumed; the host stages those 10 rows as fp16
(rel err ~1.5e-3 vs the 2e-2 gate) and upcasts the fp16 result to fp32.

Engine plan (ACT is always 1x on TRN2; DVE TT 2x / tensor_scalar 4x with
fused 2-op (op0,const0)(op1,const1); GpSimd tensor ops are slow (0.42 eff)
AND degrade concurrent DVE 2-port modes, so Pool is left idle):
  DVE  A1: t = TSP(x_ot, mult r1, add c1)  then  B1: t += x_pv (TT)
  ACT  C1: f1 = Relu(p1*t) on the 6 conv2-pv rows only (in place)
  DVE  sc4 = |r2| * f1[shared 4 rows]            (single-op TSP 4x)
  ACT  sc2 = Relu(|r2|*p1 * t[2 non-pv rows])    (pre-relu legs)
  DVE  B2: u = f1_pv -/+ sc (TT; conv2 scale pass eliminated entirely)
  C2:  G_k = w_k*relu(p2*u_k + b2):
    ACT rows: Relu(scale=|w_k|p2, bias=|w_k|b2)  (sign resolved in tree)
    DVE rows: TSP(mult w_k*p2, max/min -w_k*b2) = G_k - w_k*b2; the
      constant w_k*b2 is folded into the final bias
  DVE  tree: pairwise sign-merge TT add/sub; final TSP(add bias, max 0)
Balanced ~18.5us/tile x 8 tiles on both engines (~150us busy); DMA ~95us
fully overlapped. HW exec ~180us (run-to-run clock variance +-10%).

A hand-authored 2X_1PORT custom-DVE op for the conv inner pass (the T1
mechanism) compiles and generates tables here, but this container's
walrus rejects InstCustomDveAnt at codegen ("ISA wrong length" version
skew), so USE_CUSTOM_AFF stays off.
"""

import sys

for _p in ("/opt/trn_rl_repo",):
    if _p not in sys.path:
        sys.path.insert(0, _p)

import numpy as np

import concourse.bass as bass
import concourse.tile as tile
from concourse import mybir
from concourse.bass_utils import run_bass_kernel_spmd

N_FULL, L, MSG = 16384, 16, 688
N_CORES = 8
N_LOCAL = N_FULL // N_CORES  # 2048
P = 128
TW = 2                        # node blocks per tile
NTILES = N_LOCAL // (P * TW)  # 8
R0, NROWS = 6, 10
FB = TW * MSG                 # fused (block, msg) free dim

F16 = mybir.dt.float16
F32 = mybir.dt.float32
AF = mybir.ActivationFunctionType
OP = mybir.AluOpType

USE_CUSTOM_AFF = False  # walrus build rejects InstCustomDveAnt (ISA wrong length)
PERF_MAX_ON = False
USE_STOCK_AFF = True
N_C2_DVE = 0   # signed G rows on DVE in steady state
N_SC_DVE = 1   # sc legs on DVE


# --------------------------------------------------------------------------
# Custom 2x DVE op: out = (Src0*C0 + C1) + Src1
# --------------------------------------------------------------------------

def _aff2x_2x_uop():
    """Hand-authored 2X_1PORT program (T1 mechanism from the custom-dve
    design doc): lo element on ALU blocks 0-2, hi element on blocks 3-5
    reading the *_HI packed lanes, results packed to WR0_LO/WR0_HI."""
    from concourse.dve_uop import (
        ENABLE,
        AluInp,
        AluOp,
        DelayInp,
        InpSel,
        OutPath,
        OutSel,
        Trigger,
        UopConfig,
    )

    u = UopConfig()
    u.enable_input(InpSel.SRC_0, 0)
    u.enable_input(InpSel.SRC_1, 1)      # chain 0
    u.enable_input(InpSel.SRC_0_HI, 2)   # chain 1
    u.enable_input(InpSel.SRC_1_HI, 3)   # chain 2
    u.enable_input(InpSel.CONST_0, 4)    # chain 3 (r)
    u.enable_input(InpSel.CONST_1, 5)    # chain 4 (c)
    u.require_inp0 = ENABLE
    u.require_inp1 = ENABLE
    u.trigger = (Trigger.SRC_TENSOR_DONE, Trigger.NONE, Trigger.NONE)
    dp = u.datapath_config
    # b0: lo1 = SRC_0 * C0
    dp[0].enable_alu(AluOp.MULTIPLY, AluInp.PREV_ALU_OUT, AluInp.PREV_DELAY_3)
    dp[0].pass_through_delay(0, 1, 2, 3, 4)
    # b1: lo2 = lo1 + C1
    dp[1].enable_alu(AluOp.ADD, AluInp.PREV_ALU_OUT, AluInp.PREV_DELAY_4)
    dp[1].pass_through_delay(0, 1, 2, 3, 4)
    # b2: lo = lo2 + SRC_1
    dp[2].enable_alu(AluOp.ADD, AluInp.PREV_ALU_OUT, AluInp.PREV_DELAY_0)
    dp[2].pass_through_delay(1, 2, 3, 4)
    # b3: hi1 = SRC_0_HI * C0; park lo in chain 0
    dp[3].enable_alu(AluOp.MULTIPLY, AluInp.PREV_DELAY_1, AluInp.PREV_DELAY_3)
    dp[3].enable_delay_from_src(DelayInp.PREV_ALU_OUT, 0)
    dp[3].pass_through_delay(2, 4)
    # b4: hi2 = hi1 + C1
    dp[4].enable_alu(AluOp.ADD, AluInp.PREV_ALU_OUT, AluInp.PREV_DELAY_4)
    dp[4].pass_through_delay(0, 2)
    # b5: hi = hi2 + SRC_1_HI
    dp[5].enable_alu(AluOp.ADD, AluInp.PREV_ALU_OUT, AluInp.PREV_DELAY_2)
    dp[5].pass_through_delay(0)
    # b6: ALU <- lo (from chain 0); park hi in chain 1
    dp[6].enable_alu(AluOp.BYPASS, AluInp.PREV_DELAY_0)
    dp[6].enable_delay_from_src(DelayInp.PREV_ALU_OUT, 1)
    # b7: ALU <- lo; carry hi
    dp[7].enable_alu(AluOp.BYPASS, AluInp.PREV_ALU_OUT)
    dp[7].pass_through_delay(1)
    u.enable_output(OutSel.ALU_OUT, OutPath.WR0_LO)
    u.enable_output(OutSel.DELAY_1, OutPath.WR0_HI)
    u.validate("v3")
    return u


_AFF_OP = None


def _get_aff_op():
    """Register CONV_AFF_2X in the dve_ops registry (runtime append - the
    same operation the official flow does in-source) and seed the compile
    cache with a DveOpSpec carrying the hand-built 2x variant."""
    global _AFF_OP
    if _AFF_OP is not None:
        return _AFF_OP
    import concourse.dve_ops as dom
    from concourse.dve_spec import C0, C1, Spec, Src0, Src1
    from concourse.dve_spec import lower as dve_lower
    from concourse.dve_uop import DveOpSpec

    name = "CONV_AFF_2X"
    spec = Spec(
        body=(Src0 * C0 + C1) + Src1,
        reference=lambda in0, in1, s0, s1, imm2: (
            in0.astype(np.float32) * s0 + s1
        )
        + in1,
    )
    op = dom.DveOp(name, spec, subdim=False, uops_sha={})
    row = max(dom._SUB_OPCODE_FOR_NAME.values()) + 1
    assert row < 0x20, "no free custom-DVE opcode rows"
    dom.OPS.append(op)
    dom._SUB_OPCODE_FOR_NAME[name] = row
    dom.CUSTOM_DVE_SPECS[name] = spec
    uops_1x = dve_lower(spec, ver="v3")
    dspec = DveOpSpec(
        name=name,
        opcode=row,
        uops=uops_1x,
        uops_2x=[_aff2x_2x_uop()],
        perf_max=1,
        rd1_en=True,
    )
    dspec.validate("v3")
    dom._COMPILE_CACHE[(name, "v3")] = dspec
    _AFF_OP = op
    return op


def _split_multi_waits(nc):
    """TPB instructions encode at most ONE semaphore wait; this walrus build's
    codegen rejects instructions with more. Hoist extra waits into standalone
    EventSemaphore ops on the same (in-order) sequencer."""
    for func in nc.m.functions:
        for bb in func.blocks:
            insts = list(bb.instructions)
            if not any(
                i.sync_info is not None and len(i.sync_info.on_wait) > 1
                for i in insts
            ):
                continue
            new = []
            for inst in insts:
                si = inst.sync_info
                if si is not None and len(si.on_wait) > 1:
                    waits = list(si.on_wait)
                    for j, w in enumerate(waits[:-1]):
                        new.append(
                            mybir.InstEventSemaphore(
                                name=f"{inst.name}-hoistw{j}",
                                engine=inst.engine,
                                sync_info=mybir.SyncInfo(on_wait=[w], on_update=[]),
                            )
                        )
                    inst.sync_info = mybir.SyncInfo(
                        on_wait=[waits[-1]], on_update=list(si.on_update)
                    )
                new.append(inst)
            bb.instructions = new


def _enable_aff_perf(nc):
    """codegen packs byte36 = row | rd1_en<<5 | perf_max<<6; _custom_dve
    leaves perf_max=0 (mode Disable). Request 2X_1PORT; the engine verifies
    the mem-pattern and falls back to 1x if it doesn't qualify."""
    for func in nc.m.functions:
        for bb in func.blocks:
            for inst in bb.instructions:
                if type(inst).__name__ == "InstCustomDveAnt":
                    inst.perf_max = 1


def _conv_split(wa, wb, b):
    """Factor pre[i] = wa*in[i] + wb*in[i+2] + b as p*(in[pv] + r*in[ot] + c)
    with |r| <= 1 (pv = dominant tap), p = dominant weight, c = b/p."""
    if abs(wa) >= abs(wb):
        p, r, pv, ot = wa, (wb / wa if wa != 0.0 else 0.0), 0, 2
    else:
        p, r, pv, ot = wb, wa / wb, 2, 0
    c = b / p if p != 0.0 else 0.0
    return p, r, c, pv, ot


def build_program(w10, w11, b1, w20, w21, b2, mlp_w, mlp_b):
    nc = bass.Bass(trn_type="TRN2", name="conv_msg_agg")
    x = nc.dram_tensor("x", [N_LOCAL, NROWS, MSG], F16, kind="ExternalInput")
    out = nc.dram_tensor("out", [N_LOCAL, MSG], F16, kind="ExternalOutput")

    p1, r1, c1, pv1, ot1 = _conv_split(w10, w11, b1)
    p2, r2, c2, pv2, ot2 = _conv_split(w20, w21, b2)
    ar2 = abs(r2)
    b2op = OP.add if r2 > 0 else OP.subtract
    nzk = [k for k in range(6) if mlp_w[k] != 0.0]
    scheme_z = p1 != 0.0 and p2 != 0.0 and bool(nzk)
    # floater routing: DVE gets the final op, N_C2_DVE signed G rows (largest
    # |w| first, also guarantees a sign-+ tree root) and N_SC_DVE sc rows.
    by_mag = sorted(nzk, key=lambda k: -abs(mlp_w[k]))
    n_c2_dve = N_C2_DVE
    if nzk and not any(mlp_w[k] > 0 for k in nzk):
        n_c2_dve = max(1, N_C2_DVE)  # need a sign-+ tree root
    dve_rows = set(by_mag[:n_c2_dve]) if nzk else set()
    # conv2 bias recovered via w*relu(z+b2) = max/min(w*p2*u, -w*b2) + w*b2;
    # the constant lands in the final bias.
    fin_bias = mlp_b + sum(mlp_w[k] * b2 for k in dve_rows)

    with tile.TileContext(nc) as tc:
        with (
            tc.tile_pool(name="bias", bufs=1) as pool_b,
            tc.tile_pool(name="xin", bufs=2) as pool_x,
            tc.tile_pool(name="v1p", bufs=2) as pool_1,
            tc.tile_pool(name="v1sp", bufs=2) as pool_1s,
            tc.tile_pool(name="v2p", bufs=3) as pool_2,
            tc.tile_pool(name="outp", bufs=4) as pool_o,
        ):
            fbias = pool_b.tile([P, 1], F32, tag="fb", name="fb")
            nc.vector.memset(fbias[:], fin_bias)
            gbias = {}
            for k in nzk:
                if k not in dve_rows and abs(mlp_w[k]) * b2 != 0.0:
                    gbias[k] = pool_b.tile([P, 1], F32, tag=f"gb{k}", name=f"gb{k}")
                    nc.vector.memset(gbias[k][:], abs(mlp_w[k]) * b2)
            for it in range(NTILES):
                n0 = it * TW * P
                xt = pool_x.tile([P, NROWS, TW, MSG], F16, tag="x")
                front_split = it <= 1
                back_split = it == NTILES - 1
                if it == 0:
                    blks = [(0, 1, 0, MSG // 2), (0, 1, MSG // 2, MSG)] + [
                        (blk, blk + 1, 0, MSG) for blk in range(1, TW)
                    ]
                elif front_split or back_split:
                    blks = [(blk, blk + 1, 0, MSG) for blk in range(TW)]
                else:
                    blks = [(0, TW, 0, MSG)]
                bblks = (
                    [(blk, blk + 1) for blk in range(TW)]
                    if back_split
                    else [(0, TW)]
                )
                for lo, hi, m0, m1 in blks:
                    nc.sync.dma_start(
                        out=xt[:, :, lo:hi, m0:m1],
                        in_=x[n0 + lo * P : n0 + hi * P, :, m0:m1].rearrange(
                            "(b p) r m -> p r b m", b=hi - lo
                        ),
                    )

                # conv1 pre: t = (r1*x_ot + c1) + x_pv  (TSP 4x + TT 2x)
                v1 = pool_1.tile([P, 8, TW, MSG], F16, tag="v1")
                if p1 == 0.0:
                    nc.vector.memset(v1[:], max(b1, 0.0) / (p1 if p1 else 1.0))
                else:
                    for lo, hi, m0, m1 in blks:
                        nc.vector.tensor_scalar(
                            out=v1[:, :, lo:hi, m0:m1],
                            in0=xt[:, ot1 : ot1 + 8, lo:hi, m0:m1],
                            scalar1=r1, scalar2=c1,
                            op0=OP.mult, op1=OP.add,
                        )
                        nc.vector.tensor_tensor(
                            out=v1[:, :, lo:hi, m0:m1],
                            in0=v1[:, :, lo:hi, m0:m1],
                            in1=xt[:, pv1 : pv1 + 8, lo:hi, m0:m1],
                            op=OP.add,
                        )
                        # raw relu in place on the conv2 pv rows only;
                        # DVE in the drain phase (ACT would starve DVE)
                        if scheme_z and back_split and lo >= TW - 1:
                            nc.vector.tensor_scalar(
                                out=v1[:, pv2 : pv2 + 6, lo:hi, m0:m1],
                                in0=v1[:, pv2 : pv2 + 6, lo:hi, m0:m1],
                                scalar1=p1, scalar2=0.0,
                                op0=OP.mult, op1=OP.max,
                            )
                        elif scheme_z:
                            nc.scalar.activation(
                                out=v1[:, pv2 : pv2 + 6, lo:hi, m0:m1],
                                in_=v1[:, pv2 : pv2 + 6, lo:hi, m0:m1],
                                func=AF.Relu, bias=0.0, scale=p1,
                            )
                        else:
                            nc.scalar.activation(
                                out=v1[:, :, lo:hi, m0:m1],
                                in_=v1[:, :, lo:hi, m0:m1],
                                func=AF.Relu, bias=0.0, scale=p1,
                            )

                v1s = pool_1s.tile([P, 6, TW, MSG], F16, tag="v1s")
                v2 = pool_2.tile([P, 6, TW, MSG], F16, tag="v2")
                ot = pool_o.tile([P, TW, MSG], F16, tag="o")
                for lo, hi in bblks:
                    if not scheme_z:
                        # degenerate weights: conservative fallback
                        base = max(b1, 0.0) if p1 == 0.0 else None
                        if p2 == 0.0 or not nzk:
                            fb = max(b2, 0.0) if p2 == 0.0 else 0.0
                            tval = sum(mlp_w[k] * fb for k in nzk) + mlp_b
                            nc.vector.memset(ot[:, lo:hi, :], max(tval, 0.0))
                        else:
                            # p1 == 0: f1 rows are the constant max(b1,0)
                            f1c = max(b1, 0.0)
                            zc = w20 * f1c + w21 * f1c + b2
                            tval = sum(
                                mlp_w[k] * max(zc, 0.0) for k in nzk
                            ) + mlp_b
                            nc.vector.memset(ot[:, lo:hi, :], max(tval, 0.0))
                        nc.sync.dma_start(
                            out=out[n0 + lo * P : n0 + hi * P].rearrange(
                                "(b p) m -> p b m", b=hi - lo
                            ),
                            in_=ot[:, lo:hi, :],
                        )
                        continue

                    # conv1 sc legs: sc_j = |r2| * relu(p1 * t[ot2+j]).
                    # The 4 rows shared with the raw range are already
                    # relu'd in v1 -> one single-op 4x TSP mult on DVE;
                    # the 2 non-shared rows come from pre-relu t -> one
                    # batched ACT Relu.
                    sh0 = max(pv2, ot2)          # first shared f1 row
                    j0 = sh0 - ot2               # its v1s row
                    nc.vector.tensor_scalar_mul(
                        v1s[:, j0 : j0 + 4, lo:hi, :],
                        v1[:, sh0 : sh0 + 4, lo:hi, :],
                        ar2,
                    )
                    t0 = ot2 if ot2 < pv2 else ot2 + 4  # non-shared rows
                    jt = t0 - ot2
                    if back_split and lo >= TW - 1:
                        nc.vector.tensor_scalar(
                            out=v1s[:, jt : jt + 2, lo:hi, :],
                            in0=v1[:, t0 : t0 + 2, lo:hi, :],
                            scalar1=ar2 * p1, scalar2=0.0,
                            op0=OP.mult, op1=OP.max,
                        )
                    else:
                        nc.scalar.activation(
                            out=v1s[:, jt : jt + 2, lo:hi, :],
                            in_=v1[:, t0 : t0 + 2, lo:hi, :],
                            func=AF.Relu, bias=0.0, scale=ar2 * p1,
                        )

                    # conv2 pre: u = f1_raw_pv +/- sc  (TT 2x)
                    nc.vector.tensor_tensor(
                        out=v2[:, :, lo:hi, :],
                        in0=v1[:, pv2 : pv2 + 6, lo:hi, :],
                        in1=v1s[:, :, lo:hi, :],
                        op=b2op,
                    )

                    # C2: G_k = w_k * relu(p2*u_k + b2). In the drain
                    # phase (last tile) ACT-routed rows would starve DVE,
                    # so route everything to DVE there.
                    drain = back_split and lo >= TW - 1
                    bblk_dve = dve_rows if not drain else set(nzk)
                    bblk_fin = fin_bias if not drain else (
                        mlp_b + sum(mlp_w[k] * b2 for k in nzk)
                    )
                    terms = []  # (sign, row_ap), sign-+ rows first
                    for k in nzk:
                        dst = v2[:, k, lo:hi, :]
                        if k in bblk_dve:
                            nc.vector.tensor_scalar(
                                out=dst, in0=dst,
                                scalar1=mlp_w[k] * p2,
                                scalar2=-mlp_w[k] * b2,
                                op0=OP.mult,
                                op1=OP.max if mlp_w[k] > 0 else OP.min,
                            )
                            terms.insert(0, (1, dst))
                        else:
                            nc.scalar.activation(
                                out=dst, in_=dst, func=AF.Relu,
                                bias=gbias[k][:] if k in gbias else 0.0,
                                scale=abs(mlp_w[k]) * p2,
                            )
                            terms.append((1 if mlp_w[k] > 0 else -1, dst))

                    # tree level-1: if all 3 adjacent pairs need the SAME
                    # op, batch them into one strided TT (out rows 0,2,4).
                    # A reversed subtract pair just flips the result sign,
                    # which the tag tracks (tag = in0 row's sign).
                    if len(nzk) == 6:
                        # effective tags: DVE rows hold signed values (tag +)
                        s = [1 if (k in bblk_dve or mlp_w[k] > 0) else -1
                             for k in range(6)]
                        pair_ops = [OP.add if s[2*i] == s[2*i+1] else OP.subtract
                                    for i in range(3)]
                        batched = pair_ops[0] == pair_ops[1] == pair_ops[2]
                    else:
                        batched = False
                    if batched:
                        # strided 3-row APs must stay <=2 free dims for the
                        # 2x packed mode: fuse (blk, msg) when contiguous
                        if lo == 0 and hi == TW:
                            fl = lambda ap: ap.rearrange("p r b m -> p r (b m)")
                        else:
                            fl = lambda ap: ap.rearrange("p r b m -> p (r b) m")
                        nc.vector.tensor_tensor(
                            out=fl(v2[:, 0:6:2, lo:hi, :]),
                            in0=fl(v2[:, 0:6:2, lo:hi, :]),
                            in1=fl(v2[:, 1:6:2, lo:hi, :]),
                            op=pair_ops[0],
                        )
                        terms = [(s[2*i], v2[:, 2*i, lo:hi, :]) for i in range(3)]
                        terms.sort(key=lambda t: -t[0])
                    # pairwise sign-merge (TT add/sub on DVE)
                    while len(terms) > 1:
                        pos = [t for t in terms if t[0] > 0]
                        neg = [t for t in terms if t[0] < 0]
                        if len(neg) >= 2:
                            (sa, aa), (sb, ab) = neg[0], neg[1]
                            op = OP.add
                        elif len(pos) >= 2 and len(neg) == 0:
                            (sa, aa), (sb, ab) = pos[0], pos[1]
                            op = OP.add
                        else:  # fold the last negative into a positive
                            (sa, aa), (sb, ab) = pos[0], neg[0]
                            op = OP.subtract
                        nc.vector.tensor_tensor(out=aa, in0=aa, in1=ab, op=op)
                        terms = [
                            t for t in terms if t[1] is not aa and t[1] is not ab
                        ]
                        terms.insert(0, (sa, aa))

                    # final: out = Relu(T + bias); ACT normally, DVE in
                    # the drain phase (survivor sign + either way)
                    assert terms[0][0] > 0
                    if drain:
                        nc.vector.tensor_scalar(
                            out=ot[:, lo:hi, :], in0=terms[0][1],
                            scalar1=bblk_fin, scalar2=0.0,
                            op0=OP.add, op1=OP.max,
                        )
                    else:
                        nc.scalar.activation(
                            out=ot[:, lo:hi, :], in_=terms[0][1],
                            func=AF.Relu, bias=fbias[:], scale=1.0,
                        )
                    nc.sync.dma_start(
                        out=out[n0 + lo * P : n0 + hi * P].rearrange(
                            "(b p) m -> p b m", b=hi - lo
                        ),
                        in_=ot[:, lo:hi, :],
                    )
    _split_multi_waits(nc)
    return nc


def run(inputs, trace=False, **spmd_kwargs):
    """Build + run on 8 cores. Returns (full_output, BassKernelResults)."""
    msgs = np.asarray(inputs["messages"])
    assert msgs.shape == (N_FULL, L, MSG), msgs.shape
    xs = np.ascontiguousarray(msgs[:, R0 : R0 + NROWS, :], dtype=np.float16)

    c1w = np.asarray(inputs["conv1_w"], dtype=np.float64)
    c2w = np.asarray(inputs["conv2_w"], dtype=np.float64)
    mlw = np.asarray(inputs["mlp_w"], dtype=np.float64)
    nc = build_program(
        float(c1w[0]),
        float(c1w[1]),
        float(np.asarray(inputs["conv1_b"], dtype=np.float64)),
        float(c2w[0]),
        float(c2w[1]),
        float(np.asarray(inputs["conv2_b"], dtype=np.float64)),
        [float(v) for v in mlw],
        float(np.asarray(inputs["mlp_b"], dtype=np.float64)),
    )

    in_maps = [
        {"x": xs[i * N_LOCAL : (i + 1) * N_LOCAL]} for i in range(N_CORES)
    ]
    res = run_bass_kernel_spmd(
        nc, in_maps, core_ids=list(range(N_CORES)), trace=trace, **spmd_kwargs
    )
    full = np.concatenate([r["out"] for r in res.results], axis=0).astype(
        np.float32
    )
    return full, res


def kernel(**inputs) -> np.ndarray:
    return run(inputs, trace=False)[0]
